# revision 1
# baseline (speedup 1.0000x reference)
"""Trainium2 Bass kernel for nn_AttributeBiasLoss.

Reference computation:
    per_node = mean(sigmoid(predictions), axis=1)            # [B]
    for each attribute a: group per_node by attr_vals[:, a] (V=16 values)
    means[a, v] = mean of per_node over group (a, v)
    loss = sum over attrs of pairwise squared diffs of present group means
           / number of comparisons

Kernel strategy (data-parallel over 8 cores, batch-sharded):
  Pack z = attr + per_node into fp16 (integer part = bucket id, fraction =
  payload).  Then per attribute:
    - 16 fused tensor_scalar passes  accum = sum(min(z, v+1))   (knot family;
      first differences give S_v + C_{>=v+1})
    - 15 fused ACT passes            accum = sum(sign(z - (v+1)))  -> prefix
      counts
  All passes are single-source ops with fused per-partition accumulators --
  no one-hot materialization.  Cross-partition reduce via PE matmul with a
  ones vector, AllReduce of the tiny [1, 248] stats, and a redundant
  on-device epilogue (centered variance form) producing the scalar loss.
"""

import sys

sys.path.insert(0, "/opt/trn_rl_repo")

from contextlib import ExitStack

import numpy as np

import concourse.bacc as bacc
import concourse.bass as bass
import concourse.mybir as mybir
import concourse.tile as tile
from concourse import bass_utils
from concourse._compat import with_exitstack

# ---------------------------------------------------------------------------
# Custom two-stream knot-reduce DVE ops.
#
# The stock fused-accum tensor_scalar (TENSOR_SCALAR_CACHE_REDUCE) is locked
# at 1x = one element per lane-cycle.  A TT-shaped custom op reads TWO streams
# (rd0 + rd1), evaluates the same knot on both and accumulates the pair sum --
# one result per cycle consuming two stream elements, i.e. 2x effective rate.
#
# MIN2_KNOT_ANT:  out = min(src0, c0) + min(src1, c0); accum_out = c1 + sum(out)
# LT2_KNOT_ANT:   out = (src0 < c0) + (src1 < c0);     accum_out = c1 + sum(out)
# ---------------------------------------------------------------------------
import concourse.dve_ops as dve_ops
from concourse.dve_ops import DveOp
from concourse.dve_spec import C0, C1, Spec, Src0, Src1, lower, minn, _has_src1
from concourse.dve_uop import DveOpSpec


def _ref_body_sum_c1(body_fn):
    """Reference for Spec(body=..., accum=add, accum_init=C1)."""

    def _r(in0, in1, c0, c1, c2):
        b = body_fn(in0, in1, c0, c1, c2).astype(np.float32)
        return b, c1 + b.reshape(b.shape[0], -1).sum(axis=-1, keepdims=True)

    return _r


def _make_op(name: str, spec: Spec) -> DveOp:
    # Register the name -> opcode row first (compile() resolves it).
    row = max(dve_ops._SUB_OPCODE_FOR_NAME.values()) + 1
    assert row < 0x20
    dve_ops._SUB_OPCODE_FOR_NAME[name] = row
    # Compute the uops sha for each supported ver so the pin check passes.
    shas = {}
    for ver in ("v3", "v4"):
        uops = lower(spec, ver=ver)
        shas[ver] = DveOpSpec(
            name=name, opcode=row, uops=uops, rd1_en=_has_src1(spec)
        ).sha(ver)
    op = DveOp(name, spec, subdim=False, uops_sha=shas)
    dve_ops.OPS.append(op)
    dve_ops.CUSTOM_DVE_SPECS[name] = spec
    return op


def _ref_min2(in0, in1, c0, c1, c2):
    return np.minimum(in0.astype(np.float32), c0) + np.minimum(
        in1.astype(np.float32), c0
    )


def _ref_lt2(in0, in1, c0, c1, c2):
    return (in0.astype(np.float32) < c0).astype(np.float32) + (
        in1.astype(np.float32) < c0
    ).astype(np.float32)


_registered = {}


def get_ops():
    if not _registered:
        _registered["min2"] = _make_op(
            "MIN2_KNOT_ANT",
            Spec(
                body=minn(Src0, C0) + minn(Src1, C0),
                accum=lambda a, b: a + b,
                accum_init=C1,
                reference=_ref_body_sum_c1(_ref_min2),
            ),
        )
        _registered["lt2"] = _make_op(
            "LT2_KNOT_ANT",
            Spec(
                body=(Src0 < C0) + (Src1 < C0),
                accum=lambda a, b: a + b,
                accum_init=C1,
                reference=_ref_body_sum_c1(_ref_lt2),
            ),
        )
    return _registered


F32 = mybir.dt.float32
F16 = mybir.dt.float16
I32 = mybir.dt.int32
AF = mybir.ActivationFunctionType
OP = mybir.AluOpType

# Problem constants (hardcoded per harness contract).
B, D, A, V = 2_000_000, 8, 8, 16
NCORES = 8
ROWS_PER_CORE = B // NCORES  # 250_000

PAD_ATTR = 16  # out-of-range bucket: contributes +1 to every min-knot, 0 counts


def plan(rows_per_core):
    """Choose per-partition columns (CP) and subchunking for a shard size."""
    # rows laid out partition-major: partition p owns CP consecutive rows.
    sub = 8
    cp = -(-rows_per_core // (128 * sub)) * sub  # round up to multiple of sub
    rows_pad = 128 * cp
    return cp, sub, rows_pad


CP, SUB, ROWS_PAD = plan(ROWS_PER_CORE)  # 1960, 10, 250_880
CSUB = CP // SUB


@with_exitstack
def emit_kernel(
    ctx: ExitStack,
    tc: tile.TileContext,
    pred_d,  # DRAM [ROWS_PAD, D] f32
    attr_d,  # DRAM [A, ROWS_PAD] i32 (attribute-major, host-transposed)
    loss_d,  # DRAM [1, 1] f32
    cp=CP,
    sub=SUB,
    n_cores=NCORES,
    rows_real_core=ROWS_PER_CORE,
):
    nc = tc.nc
    csub = cp // sub
    rows_pad = 128 * cp
    npad_tot = float((rows_pad - rows_real_core) * n_cores)  # pads per attr, global
    n_real = float(rows_real_core * n_cores)  # real rows per attr, global
    n_tot_pad = float(rows_pad * n_cores)

    io = ctx.enter_context(tc.tile_pool(name="io", bufs=3))
    zp = ctx.enter_context(tc.tile_pool(name="z", bufs=1))
    pnp = ctx.enter_context(tc.tile_pool(name="pn", bufs=3))
    accp = ctx.enter_context(tc.tile_pool(name="acc", bufs=1))
    junkp = ctx.enter_context(tc.tile_pool(name="junk", bufs=1))
    smallp = ctx.enter_context(tc.tile_pool(name="small", bufs=1))
    psump = ctx.enter_context(tc.tile_pool(name="ps", bufs=1, space="PSUM"))
    dramp = ctx.enter_context(tc.tile_pool(name="dram", bufs=1, space="DRAM"))

    # z: attribute-major fp32 stream: col a*cp + (s*csub + c).
    # fp32 costs the same as fp16 here (knot ops are 1 result/cycle on both
    # engines) and removes the payload quantization error entirely.
    z16 = zp.tile([128, A * cp], F32)
    z16_ac = z16.rearrange("p (a c) -> p a c", a=A)  # [128, A, cp], inner step 1

    # per-(attr, knot) accumulator columns:
    #   cols [0, 128)   : M_v  = sum min(z, v+1), col = a*16 + v
    #   cols [128, 248) : G_v  = sum sign(z - (v+1)), col = 128 + a*15 + v
    NACC = A * V + A * (V - 1)  # 248
    A_DVE = 6  # knot sets: attrs [0, A_DVE) on DVE min2; rest on ACT Relu
    A_CNT = 4  # count sets: attrs [0, A_CNT) on DVE lt2; rest on ACT Sign
    KC = 2  # plus attr 6 counts v < KC on DVE lt2 (fine-grain engine balance)
    acc = accp.tile([128, NACC], F32)

    junk_v = junkp.tile([128, cp], F32, tag="junk_v")
    junk_a = junkp.tile([128, cp], F32, tag="junk_a")

    # per-knot bias constants for the ACT passes: Sign uses -(v+1), Relu uses -v
    bias_t = smallp.tile([128, V + 1], F32, name="bias")
    for v in range(V + 1):
        nc.vector.memset(bias_t[:, v : v + 1], -float(v))

    # Warm up the collective engine early (channel setup dominates the first
    # collective); no data dependencies so it overlaps the main compute.
    warm_in = dramp.tile([1, NACC], F32, name="warm_in")
    warm_out = dramp.tile([1, NACC], F32, name="warm_out")
    warm_s = smallp.tile([1, NACC], F32, name="warm_s")
    nc.vector.memset(warm_s[:], 0.0)
    nc.sync.dma_start(warm_in[:], warm_s[:])
    nc.gpsimd.collective_compute(
        "AllReduce",
        OP.add,
        replica_groups=[list(range(n_cores))],
        ins=[warm_in.opt()],
        outs=[warm_out.opt()],
    )

    pred_v = pred_d.rearrange("(p s c) d -> s p (c d)", p=128, s=sub)
    attr_v = attr_d.rearrange("a (p s c) -> s p a c", p=128, s=sub)

    for s in range(sub):
        pred_t = io.tile([128, csub * D], F32, tag="pred")
        nc.sync.dma_start(pred_t[:], pred_v[s])
        attr_t = io.tile([128, csub * A], I32, tag="attr")
        nc.sync.dma_start(attr_t.rearrange("p (a c) -> p a c", a=A), attr_v[s])

        sig_t = io.tile([128, csub * D], F32, tag="sig")
        nc.scalar.activation(sig_t[:], pred_t[:], AF.Sigmoid)
        t1 = pnp.tile([128, csub], F32, tag="t1")
        nc.vector.tensor_reduce(
            t1[:],
            sig_t.rearrange("p (c d) -> p c d", d=D),
            op=OP.add,
            axis=mybir.AxisListType.X,
        )

        # z[p, a, s*csub + c] = t1[p, c] * 0.125 + attr[p, c, a]   (fp16 out)
        # Iterate (a outer, c inner) so the z16 write is inner-contiguous.
        nc.vector.scalar_tensor_tensor(
            out=z16_ac[:, :, s * csub : (s + 1) * csub],
            in0=t1[:].broadcast_to([128, csub, A]).rearrange("p c a -> p a c"),
            scalar=1.0 / D,
            in1=attr_t.rearrange("p (a c) -> p a c", a=A),
            op0=OP.mult,
            op1=OP.add,
        )

    # Knot passes, balanced across DVE and ACT at whole-attribute granularity
    # (families can't mix within an attr -- the recovery differences
    # consecutive knots of one family):
    #   attrs < A_DVE:  DVE two-stream custom ops: min2 knots (M_v) and
    #                   lt2 counts (C_v raw), ~632ns/pass
    #   attrs >= A_DVE: ACT Relu knots (R_v) + Sign counts (G_v), ~2.2us/pass
    # A_DVE = 6 balances 186*0.72us (DVE) vs 62*2.2us (ACT).
    cops = get_ops()
    H = cp // 2
    junk_h = junkp.tile([128, H], F32, name="junk_h")
    for a in range(A):
        zs = z16[:, a * cp : (a + 1) * cp]
        z0 = z16[:, a * cp : a * cp + H]
        z1 = z16[:, a * cp + H : (a + 1) * cp]
        if a < A_DVE:
            for v in range(V):
                nc.vector._custom_dve(
                    cops["min2"],
                    out=junk_h[:],
                    in0=z0,
                    in1=z1,
                    s0=float(v + 1),
                    s1=0.0,
                    accum_out=acc[:, a * V + v : a * V + v + 1],
                )
        else:
            for v in range(V):
                nc.scalar.activation(
                    junk_a[:],
                    zs,
                    AF.Relu,
                    bias=bias_t[:, v : v + 1],
                    accum_out=acc[:, a * V + v : a * V + v + 1],
                )
        for v in range(V - 1):
            col = A * V + a * (V - 1) + v
            if a < A_CNT or (a == 6 and v < KC):
                nc.vector._custom_dve(
                    cops["lt2"],
                    out=junk_h[:],
                    in0=z0,
                    in1=z1,
                    s0=float(v + 1),
                    s1=0.0,
                    accum_out=acc[:, col : col + 1],
                )
            else:
                nc.scalar.activation(
                    junk_a[:],
                    zs,
                    AF.Sign,
                    bias=bias_t[:, v + 1 : v + 2],
                    accum_out=acc[:, col : col + 1],
                )

    # Difference the knot blocks along v per partition (keeps the globally
    # reduced values small for fp32 accuracy):
    #   M family (a < A_DVE):  dM[a, 0] = M[a, 0]; dM[a, v] = M[a,v] - M[a,v-1]
    #   R family (a >= A_DVE): dR[a,15] = R[a,15]; dR[a, v] = R[a,v] - R[a,v+1]
    # Both give  d[a, v] = S_v + C_{>=v+1} + npad.
    dacc = accp.tile([128, NACC], F32, name="dacc")
    accS = acc[:, 0 : A * V].rearrange("p (a v) -> p a v", a=A)
    dS = dacc[:, 0 : A * V].rearrange("p (a v) -> p a v", a=A)
    nc.vector.tensor_tensor(
        out=dS[:, 0:A_DVE, 1:V],
        in0=accS[:, 0:A_DVE, 1:V],
        in1=accS[:, 0:A_DVE, 0 : V - 1],
        op=OP.subtract,
    )
    nc.vector.tensor_copy(dS[:, 0:A_DVE, 0:1], accS[:, 0:A_DVE, 0:1])
    nc.vector.tensor_tensor(
        out=dS[:, A_DVE:A, 0 : V - 1],
        in0=accS[:, A_DVE:A, 0 : V - 1],
        in1=accS[:, A_DVE:A, 1:V],
        op=OP.subtract,
    )
    nc.vector.tensor_copy(dS[:, A_DVE:A, V - 1 : V], accS[:, A_DVE:A, V - 1 : V])
    nc.vector.tensor_copy(dacc[:, A * V : NACC], acc[:, A * V : NACC])

    # Cross-partition reduce: ones[128,1].T @ dacc -> [1, NACC]
    ones_t = smallp.tile([128, 1], F32, name="ones")
    nc.vector.memset(ones_t[:], 1.0)
    red_ps = psump.tile([1, NACC], F32)
    nc.tensor.matmul(red_ps[:], lhsT=ones_t[:], rhs=dacc[:], start=True, stop=True)

    core_stats = smallp.tile([1, NACC], F32, name="core_stats")
    nc.vector.tensor_copy(core_stats[:], red_ps[:])

    # AllReduce the tiny stats vector across the 8 cores.
    cc_in = dramp.tile([1, NACC], F32, name="cc_in")
    cc_out = dramp.tile([1, NACC], F32, name="cc_out")
    nc.sync.dma_start(cc_in[:], core_stats[:])
    nc.gpsimd.collective_compute(
        "AllReduce",
        OP.add,
        replica_groups=[list(range(n_cores))],
        ins=[cc_in.opt()],
        outs=[cc_out.opt()],
    )
    g = smallp.tile([1, NACC], F32, name="g")
    nc.sync.dma_start(g[:], cc_out[:])

    # ---------------- epilogue (tiny, partition 0, redundant per core) -------
    ep = ctx.enter_context(tc.tile_pool(name="ep", bufs=1))

    gM = g[:, 0 : A * V].rearrange("p (a v) -> p a v", a=A)  # dM global
    gG = g[:, A * V : NACC].rearrange("p (a v) -> p a v", a=A)  # G_v global

    # Prefix counts C[a, v] for v<15; C[a, 15] = n_real.
    #   attrs < A_DVE:  raw lt2 sums: C = G
    #   attrs >= A_DVE: Sign-coded: C = (n_tot_pad - G) / 2
    C = ep.tile([1, A * V], F32, name="C").rearrange("p (a v) -> p a v", a=A)
    A_DVE, A_CNT = 6, 4
    KC = 2
    nc.vector.tensor_copy(C[:, 0:A_CNT, 0 : V - 1], gG[:, 0:A_CNT, :])
    nc.vector.tensor_scalar(
        out=C[:, A_CNT:A, 0 : V - 1],
        in0=gG[:, A_CNT:A, :],
        scalar1=-0.5,
        scalar2=n_tot_pad / 2.0,
        op0=OP.mult,
        op1=OP.add,
    )
    nc.vector.tensor_copy(C[:, 6:7, 0:KC], gG[:, 6:7, 0:KC])
    nc.vector.memset(C[:, :, V - 1 : V], n_real)

    # n[a, 0] = C[a, 0]; n[a, v] = C[a, v] - C[a, v-1]
    n_t = ep.tile([1, A * V], F32, name="n").rearrange("p (a v) -> p a v", a=A)
    nc.vector.tensor_tensor(
        out=n_t[:, :, 1:V],
        in0=C[:, :, 1:V],
        in1=C[:, :, 0 : V - 1],
        op=OP.subtract,
    )
    nc.vector.tensor_copy(n_t[:, :, 0:1], C[:, :, 0:1])

    # S[a, v] = d[a, v] + C[a, v] - n_real - npad_tot
    S = ep.tile([1, A * V], F32, name="S").rearrange("p (a v) -> p a v", a=A)
    nc.vector.tensor_tensor(out=S, in0=gM, in1=C, op=OP.add)
    nc.vector.tensor_scalar(
        out=S, in0=S, scalar1=-(n_real + npad_tot), scalar2=None, op0=OP.add
    )
    # R-family top knot: pads sit at z = 16.5 exactly (zero-padded preds ->
    # sigmoid 0.5), contributing 1.5 per pad to dR[15] instead of 1.
    nc.vector.tensor_scalar(
        out=S[:, A_DVE:A, V - 1 : V],
        in0=S[:, A_DVE:A, V - 1 : V],
        scalar1=-0.5 * npad_tot,
        scalar2=None,
        op0=OP.add,
    )

    # m = S / max(n, 1)
    nmax = ep.tile([1, A * V], F32, name="nmax")
    nc.vector.tensor_scalar(
        out=nmax[:], in0=n_t.rearrange("p a v -> p (a v)"), scalar1=1.0, scalar2=None,
        op0=OP.max,
    )
    rn = ep.tile([1, A * V], F32, name="rn")
    nc.vector.reciprocal(rn[:], nmax[:])
    m = ep.tile([1, A * V], F32, name="m").rearrange("p (a v) -> p a v", a=A)
    nc.vector.tensor_tensor(
        out=m, in0=S, in1=rn.rearrange("p (a v) -> p a v", a=A), op=OP.mult
    )

    # present mask & per-attr stats
    p_t = ep.tile([1, A * V], F32, name="p").rearrange("p (a v) -> p a v", a=A)
    nc.vector.tensor_scalar(
        out=p_t, in0=n_t, scalar1=0.5, scalar2=None, op0=OP.is_ge
    )
    k_t = ep.tile([1, A], F32, name="k")
    nc.vector.tensor_reduce(k_t[:], p_t, op=OP.add, axis=mybir.AxisListType.X)

    mp = ep.tile([1, A * V], F32, name="mp").rearrange("p (a v) -> p a v", a=A)
    nc.vector.tensor_tensor(out=mp, in0=m, in1=p_t, op=OP.mult)
    ms = ep.tile([1, A], F32, name="ms")
    nc.vector.tensor_reduce(ms[:], mp, op=OP.add, axis=mybir.AxisListType.X)

    kmax = ep.tile([1, A], F32, name="kmax")
    nc.vector.tensor_scalar(
        out=kmax[:], in0=k_t[:], scalar1=1.0, scalar2=None, op0=OP.max
    )
    rk = ep.tile([1, A], F32, name="rk")
    nc.vector.reciprocal(rk[:], kmax[:])
    mu = ep.tile([1, A], F32, name="mu")
    nc.vector.tensor_tensor(out=mu[:], in0=ms[:], in1=rk[:], op=OP.mult)

    # d = (m - mu) * present ; q = sum_v d^2 ; contrib = k * q
    dtile = ep.tile([1, A * V], F32, name="d").rearrange("p (a v) -> p a v", a=A)
    nc.vector.scalar_tensor_tensor(
        out=dtile,
        in0=mu[:].broadcast_to([1, A, V]),
        scalar=-1.0,
        in1=m,
        op0=OP.mult,
        op1=OP.add,
    )
    nc.vector.tensor_tensor(out=dtile, in0=dtile, in1=p_t, op=OP.mult)
    d2 = ep.tile([1, A * V], F32, name="d2").rearrange("p (a v) -> p a v", a=A)
    nc.vector.tensor_tensor(out=d2, in0=dtile, in1=dtile, op=OP.mult)
    q_t = ep.tile([1, A], F32, name="q")
    nc.vector.tensor_reduce(q_t[:], d2, op=OP.add, axis=mybir.AxisListType.X)

    contrib = ep.tile([1, A], F32, name="contrib")
    nc.vector.tensor_tensor(out=contrib[:], in0=k_t[:], in1=q_t[:], op=OP.mult)
    tot = ep.tile([1, 1], F32, name="tot")
    nc.vector.tensor_reduce(tot[:], contrib[:], op=OP.add, axis=mybir.AxisListType.X)

    # ncomp = sum_a k(k-1)/2
    kk = ep.tile([1, A], F32, name="kk")
    nc.vector.scalar_tensor_tensor(
        out=kk[:], in0=k_t[:], scalar=-1.0, in1=k_t[:], op0=OP.add, op1=OP.mult
    )
    ncomp = ep.tile([1, 1], F32, name="ncomp")
    nc.vector.tensor_reduce(ncomp[:], kk[:], op=OP.add, axis=mybir.AxisListType.X)
    nc.vector.tensor_scalar(
        out=ncomp[:], in0=ncomp[:], scalar1=0.5, scalar2=None, op0=OP.mult
    )

    # loss = (ncomp > 0) * tot / max(ncomp, 0.5)
    ncm = ep.tile([1, 1], F32, name="ncm")
    nc.vector.tensor_scalar(
        out=ncm[:], in0=ncomp[:], scalar1=0.5, scalar2=None, op0=OP.max
    )
    rnc = ep.tile([1, 1], F32, name="rnc")
    nc.vector.reciprocal(rnc[:], ncm[:])
    mask = ep.tile([1, 1], F32, name="mask")
    nc.vector.tensor_scalar(
        out=mask[:], in0=ncomp[:], scalar1=0.25, scalar2=None, op0=OP.is_ge
    )
    res = ep.tile([1, 1], F32, name="res")
    nc.vector.tensor_tensor(out=res[:], in0=tot[:], in1=rnc[:], op=OP.mult)
    nc.vector.tensor_tensor(out=res[:], in0=res[:], in1=mask[:], op=OP.mult)

    nc.sync.dma_start(loss_d[:], res[:])


def build(cp=CP, sub=SUB, n_cores=NCORES, rows_real_core=ROWS_PER_CORE):
    rows_pad = 128 * cp
    nc = bacc.Bacc(
        "TRN2", target_bir_lowering=False, debug=False, num_devices=n_cores
    )
    pred_d = nc.dram_tensor("pred", [rows_pad, D], F32, kind="ExternalInput").ap()
    attr_d = nc.dram_tensor("attr", [A, rows_pad], I32, kind="ExternalInput").ap()
    loss_d = nc.dram_tensor("loss", [1, 1], F32, kind="ExternalOutput").ap()
    with tile.TileContext(nc) as tc:
        emit_kernel(
            tc,
            pred_d,
            attr_d,
            loss_d,
            cp=cp,
            sub=sub,
            n_cores=n_cores,
            rows_real_core=rows_real_core,
        )
    nc.compile()
    return nc


def shard_inputs(predictions, attr_vals, n_cores=NCORES, rows_pad=ROWS_PAD):
    rows = predictions.shape[0] // n_cores
    in_maps = []
    for c in range(n_cores):
        p = predictions[c * rows : (c + 1) * rows]
        a = attr_vals[c * rows : (c + 1) * rows]
        pad = rows_pad - rows
        if pad:
            p = np.concatenate([p, np.zeros((pad, D), np.float32)], axis=0)
            a = np.concatenate(
                [a, np.full((pad, A), PAD_ATTR, np.int32)], axis=0
            )
        in_maps.append(
            {
                "pred": np.ascontiguousarray(p),
                "attr": np.ascontiguousarray(a.T),
            }
        )
    return in_maps


_NC_CACHE = {}


def kernel(predictions: np.ndarray, attr_vals: np.ndarray) -> np.ndarray:
    predictions = np.asarray(predictions, np.float32)
    attr_vals = np.asarray(attr_vals, np.int32)
    if "nc" not in _NC_CACHE:
        _NC_CACHE["nc"] = build()
    nc = _NC_CACHE["nc"]
    in_maps = shard_inputs(predictions, attr_vals)
    res = bass_utils.run_bass_kernel_spmd(nc, in_maps, list(range(NCORES)))
    return np.float32(res.results[0]["loss"][0, 0])

